# revision 21
# baseline (speedup 1.0000x reference)
"""Capsule-routing kernel for 8 TRN2 NeuronCores.

Strategy (n-sharded, u_hat never materialized):
  u_hat[b,n,c,d] = sum_i u[b,n,i] W[n,c,i,d] is only ever needed inside two
  contractions per routing iteration, both of which factor through W:
    (A) logits[b,n,c] = sum_d u_hat . Vacc  = sum_i u[b,n,i] * WV[b,n,c,i]
        with WV[b,n,c,i] = sum_d W[n,c,i,d] Vacc[b,c,d]   (PE matmul, p=d,
        float32r, out packed [(g4,b32); nl,i] for full-width DVE work)
    (B) s[b,c,d] = sum_n coup . u_hat = sum_{n,i} (coup[b,n,c] u[b,n,i]) W[n,c,i,d]
        p = (n16,i8) packed chunks, m = (c,d) halves of 80, f = (c',b) = 320,
        keeping only the diagonal c==c' blocks. 10x redundant f-columns, but
        only 64 matmul instructions per round at float32r 1 cycle/row - the
        NX issue floor (~97ns/instruction) dominates, so instruction count
        is the currency. Round 0 has constant coupling (1/C), so its rhs is
        u directly (no waste, fp32, 64 matmuls).
  coup lives as [nl128; c,g,b] after a PE transpose; a constant replication
  matmul (R_w) re-packs it to [(n16,i); c,b] per 16-capsule chunk for (B).
  Each core owns 512 of the 4096 input capsules. Per-round global sums are
  AllGathered (20KB) and reduced on-chip; final reduce + squash on host.

Layouts (host-prepared; partition dim first):
  WA [128=(g4,d32) ; c10, nl128, i8]  d zero-padded 16->32
  WP [128=(n16,i8) ; ch32, c10, d16]  packed W, chunk = 16 capsules
  UP [128=(n16,i8) ; ch32, b32]
  u4 [128=(g4,b32) ; nl128, i8]
  RW [128=nl       ; w8, m128]  R_w[nl, (n16,i)] = (nl == w*16+n16)
  isid [128=(k8,d16); d16]      identity stack for the 8-way gathered reduce
  rep16 [16=d      ; g4, d32]   replication matrix for vrep update
"""

import sys
import ml_dtypes
import numpy as np

sys.path.insert(0, "/opt/trn_rl_repo")

from contextlib import ExitStack

import concourse.bass as bass
import concourse.tile as tile
from concourse import bacc, mybir, masks
from concourse.bass_utils import run_bass_kernel_spmd

F32 = mybir.dt.float32
F32R = mybir.dt.float32r
BF16 = mybir.dt.bfloat16
AX = mybir.AxisListType
ALU = mybir.AluOpType
ACTF = mybir.ActivationFunctionType

B, N, C, DI, DV = 32, 4096, 10, 8, 16
NCORES = 8
NL = N // NCORES          # 512 capsules per core
G = 4                     # n-groups per core
NLG = NL // G             # 128 capsules per group
NCH = NL * DI // 128      # 32 packed (n16,i) chunks
NUM_ROUTING = 3
EPS = 1e-7


def _r(ap):
    return ap.bitcast(F32R)


def _body(ctx, tc, dins, out_d, taps=None, repeat=1):
    nc = tc.nc

    def tap(name, ap):
        if taps is not None and name in taps:
            if ap.dtype != F32:
                ap = ap.bitcast(F32)
            nc.sync.dma_start(taps[name].ap(), ap)

    consts = ctx.enter_context(tc.tile_pool(name="consts", bufs=1))
    persist = ctx.enter_context(tc.tile_pool(name="persist", bufs=1))
    work = ctx.enter_context(tc.tile_pool(name="work", bufs=2))
    psum_big = ctx.enter_context(tc.tile_pool(name="psum_big", bufs=2, space="PSUM"))
    psum_sm = ctx.enter_context(tc.tile_pool(name="psum_sm", bufs=2, space="PSUM"))
    dram = ctx.enter_context(tc.tile_pool(name="dram", bufs=1, space="DRAM"))

    # ---- constant / input loads ------------------------------------------
    wa = consts.tile([128, C, NLG, DI], BF16)
    wp = consts.tile([128, NCH, C, DV], F32R)
    up = consts.tile([128, NCH, B], F32R)
    u4 = consts.tile([G * B, NLG, DI], F32)
    rw = consts.tile([NLG, 8, 128], F32R)
    isid = consts.tile([128, DV], F32)
    rep16 = consts.tile([DV, G * 32], F32)
    ident = consts.tile([128, 128], F32)
    ones16 = consts.tile([DV, 1], F32)

    nc.sync.dma_start(up[:], dins["up"].ap().rearrange(
        "p (ch b) -> p ch b", ch=NCH).bitcast(F32R))
    nc.sync.dma_start(wp[:], dins["wp"].ap().rearrange(
        "p (ch c d) -> p ch c d", ch=NCH, c=C).bitcast(F32R))
    nc.sync.dma_start(u4[:], dins["u4"].ap().rearrange(
        "p (nl i) -> p nl i", nl=NLG))
    nc.sync.dma_start(rw[:], dins["rw"].ap().rearrange(
        "p (w m) -> p w m", w=8).bitcast(F32R))
    nc.sync.dma_start(wa[:], dins["wa"].ap().rearrange(
        "p (c nl i) -> p c nl i", c=C, nl=NLG))
    nc.sync.dma_start(isid[:], dins["isid"].ap())
    nc.sync.dma_start(rep16[:], dins["rep16"].ap())
    masks.make_identity(nc, ident[:])
    nc.gpsimd.memset(ones16[:], 1.0)

    # constant APs for activation bias operands
    czero = consts.tile([128, 1], F32)
    nc.gpsimd.memset(czero[:], 0.0)
    nc.const_aps.aps[(F32, 0.0)] = czero[:]
    ceps = consts.tile([128, 1], F32)
    nc.gpsimd.memset(ceps[:], EPS)
    nc.const_aps.aps[(F32, EPS)] = ceps[:]

    # persistent accumulators
    vrep = persist.tile([128, C, B], F32)       # [(g,d32); c,b] Vacc replicated
    vrep_r = persist.tile([128, C, B], BF16)    # bf16 copy for the PE (A) pass
    nc.gpsimd.memset(vrep[:], 0.0)

    coup_t = persist.tile([NLG, C, G, B], F32R)  # [nl; c, g, b]
    cup = persist.tile([128, NCH, C, B], F32R)   # [(n16,i); ch, c, b] packed cu
    logits = persist.tile([G * B, C, NLG], F32)

    def squash_update(stot_sb):
        """squash scale from stot [16; (c,b)], fold scale*stot into vrep."""
        sq = work.tile([DV, C * B], F32, tag="sq")
        nc.scalar.square(sq[:], stot_sb[:])
        ps_s2 = psum_sm.tile([1, C * B], F32, tag="ps_misc")
        nc.tensor.matmul(ps_s2[:], lhsT=ones16[:], rhs=sq[:], start=True, stop=True)
        s2 = work.tile([1, C * B], F32, tag="s2")
        nc.vector.tensor_copy(s2[:], ps_s2[:])
        t1 = work.tile([1, C * B], F32, tag="t1")
        nc.vector.tensor_scalar_add(t1[:], s2[:], 1.0)
        r1 = work.tile([1, C * B], F32, tag="r1")
        nc.vector.reciprocal(r1[:], t1[:])
        rt = work.tile([1, C * B], F32, tag="rt")
        nc.scalar.activation(rt[:], s2[:], ACTF.Sqrt, bias=EPS)
        r2 = work.tile([1, C * B], F32, tag="r2")
        nc.vector.reciprocal(r2[:], rt[:])
        sc = work.tile([1, C * B], F32, tag="sc")
        nc.vector.tensor_tensor(out=sc[:], in0=s2[:], in1=r1[:], op=ALU.mult)
        nc.vector.tensor_tensor(out=sc[:], in0=sc[:], in1=r2[:], op=ALU.mult)
        scale_rep = work.tile([128, C * B], F32, tag="scale_rep")
        nc.gpsimd.partition_broadcast(scale_rep[:], sc[:])

        ps_srep = psum_sm.tile([128, C * B], F32, tag="ps_misc")
        nc.tensor.matmul(ps_srep[:], lhsT=rep16[:], rhs=stot_sb[:],
                         start=True, stop=True)
        vinc = work.tile([128, C * B], F32, tag="vinc")
        nc.vector.tensor_tensor(out=vinc[:], in0=ps_srep[:], in1=scale_rep[:],
                                op=ALU.mult)
        nc.vector.tensor_tensor(
            out=vrep[:].rearrange("p c b -> p (c b)"),
            in0=vrep[:].rearrange("p c b -> p (c b)"),
            in1=vinc[:], op=ALU.add)
        nc.scalar.copy(vrep_r[:].rearrange("p c b -> p (c b)"),
                       vrep[:].rearrange("p c b -> p (c b)"))

    def reduce8(gath):
        """sum the 8 gathered partials [(k,d); (c,b)] -> stot_sb [16; (c,b)]."""
        ps_tot = psum_sm.tile([DV, C * B], F32, tag="ps_misc")
        nc.tensor.matmul(ps_tot[:], lhsT=isid[:], rhs=gath[:], start=True, stop=True)
        stot_sb = work.tile([DV, C * B], F32, tag="stot_sb")
        nc.vector.tensor_copy(stot_sb[:], ps_tot[:])
        return stot_sb

    def gather(write_part, rnd, rep_i):
        """AllGather the partial sum in [d; (c,b)] DRAM layout, reduce."""
        d_part = dram.tile([DV, C * B], F32, tag=f"dpart{rnd}_{rep_i}")
        d_gath = dram.tile([NCORES * DV, C * B], F32, tag=f"dgath{rnd}_{rep_i}")
        write_part(d_part)
        nc.gpsimd.collective_compute(
            "AllGather", ALU.bypass, replica_groups=[list(range(NCORES))],
            ins=[d_part[:].opt()], outs=[d_gath[:].opt()])
        gath = work.tile([128, C * B], F32, tag="gath")
        nc.sync.dma_start(gath[:], d_gath[:])
        return reduce8(gath[:])

    GROUPS = ((0, 8), (8, 2))   # (c_base, n_capsules) -> m = 128 / 32

    def bsum(rhs_for):
        """(B): psum groups [(c,d16); f], PSUM-accumulated over the 32
        packed chunks. Group 0 = capsules 0-7 (m=128), group 1 = 8-9."""
        groups = []
        fdim = rhs_for(0).free_size()
        for c0, nc_ in GROUPS:
            ph = psum_big.tile([16 * nc_, fdim], F32, tag=f"ps_b{c0}", bufs=1)
            for ch in range(NCH):
                nc.tensor.matmul(
                    ph[:],
                    lhsT=wp[:, ch, c0:c0 + nc_, :].rearrange(
                        "p c d -> p (c d)"),
                    rhs=rhs_for(ch),
                    start=(ch == 0), stop=(ch == NCH - 1),
                    skip_group_check=True,
                )
            groups.append(ph)
        return groups

    def diag_out(groups, dst_for):
        """Copy psum groups to SBUF, then DMA the diagonal [16;32] blocks to
        DRAM. dst_for(c) gives the [16, 32] DRAM destination AP."""
        for (c0, nc_), grp in zip(GROUPS, groups):
            s_f = work.tile([16 * nc_, C * B], F32, tag=f"s_f{c0}")
            nc.scalar.copy(s_f[:], grp[:])
            for cl in range(nc_):
                c = c0 + cl
                nc.sync.dma_start(
                    dst_for(c),
                    s_f[16 * cl:16 * (cl + 1), c * B:(c + 1) * B])

    def round0(rep_i):
        groups = bsum(lambda ch: up[:, ch, :])
        s_h = []
        for (c0, nc_), grp in zip(GROUPS, groups):
            t = work.tile([16 * nc_, B], F32, tag=f"s_h{c0}")
            nc.scalar.activation(t[:], grp[:], ACTF.Copy, scale=1.0 / C)
            s_h.append(t)

        def write_part(d_part):
            dp = d_part[:].rearrange("d (c b) -> d c b", c=C)
            for (c0, nc_), t in zip(GROUPS, s_h):
                nc.sync.dma_start(
                    dp[:, c0:c0 + nc_, :].transpose([1, 0, 2]),
                    t[:])
        return gather(write_part, 0, rep_i)

    def round12(rnd, rep_i, last):
        # ---- (A): WV then logits -----------------------------------------
        for c in range(C):
            ps_wv = psum_big.tile([128, NLG, DI], F32, tag="ps_wv")
            for g in range(G):
                for h in range(2):
                    nc.tensor.matmul(
                        ps_wv[32 * g:32 * (g + 1),
                              64 * h:64 * (h + 1), :].rearrange(
                            "p nl i -> p (nl i)"),
                        lhsT=vrep_r[32 * g:32 * (g + 1), c, :],
                        rhs=wa[32 * g:32 * (g + 1), c,
                               64 * h:64 * (h + 1), :],
                        start=True, stop=True,
                        tile_position=(32 * g, 32 * g),
                    )
            wvu = work.tile([128, NLG, DI], F32, tag="wvu")
            nc.vector.tensor_tensor(out=wvu[:], in0=ps_wv[:], in1=u4[:],
                                    op=ALU.mult)
            nc.vector.tensor_reduce(
                logits[:, c, :], wvu[:], axis=AX.X, op=ALU.add)

        # ---- softmax over c ----------------------------------------------
        expd = work.tile([G * B, C, NLG], F32, tag="expd")
        nc.scalar.activation(expd[:], logits[:], ACTF.Exp)
        den = work.tile([G * B, NLG], F32, tag="den")
        nc.vector.tensor_reduce(
            den[:], expd[:].transpose([0, 2, 1]), axis=AX.X, op=ALU.add)
        rden = work.tile([G * B, NLG], F32, tag="rden")
        nc.vector.reciprocal(rden[:], den[:])
        if rnd == 1:
            tap("logits1", logits[:].rearrange("p c n -> p (c n)"))
            tap("den1", den[:])
        coupq = work.tile([G * B, C, NLG], F32, tag="coupq")
        nc.vector.tensor_tensor(
            out=coupq[:], in0=expd[:],
            in1=rden[:].unsqueeze(1).broadcast_to((G * B, C, NLG)),
            op=ALU.mult)

        # ---- transpose coup to [nl; c,g,b] -------------------------------
        for c in range(C):
            ps_tr = psum_sm.tile([NLG, G * B], F32, tag="ps_misc")
            nc.tensor.transpose(ps_tr[:], coupq[:, c, :], ident[:])
            nc.scalar.copy(
                coup_t[:, c, :, :].rearrange("p g b -> p (g b)"), ps_tr[:])
        if rnd == 1:
            tap("coupt1", coup_t[:].rearrange("p c g b -> p (c g b)"))

        # ---- repack coup to [(n16,i); ch, c, b] and fold in u ------------
        for ch in range(NCH):
            g, w = ch // 8, ch % 8
            ps_rep = psum_sm.tile([128, C, B], F32, tag="ps_misc")
            nc.tensor.matmul(
                ps_rep[:].rearrange("p c b -> p (c b)"),
                lhsT=rw[:, w, :],
                rhs=coup_t[:, :, g, :],
                start=True, stop=True,
            )
            nc.vector.tensor_tensor(
                out=cup[:, ch, :, :],
                in0=ps_rep[:],
                in1=up[:, ch, :].bitcast(F32).unsqueeze(1).broadcast_to(
                    (128, C, B)),
                op=ALU.mult)

        # ---- (B) diagonal-waste weighted sum -----------------------------
        groups = bsum(lambda ch: cup[:, ch, :, :])

        if last:
            diag_out(groups,
                     lambda c: out_d.ap()[16 * c:16 * (c + 1), :])
            return None

        def write_part(d_part):
            dp = d_part[:].rearrange("d (c b) -> d c b", c=C)
            diag_out(groups, lambda c: dp[:, c, :])
        return gather(write_part, rnd, rep_i)

    for rep_i in range(repeat):
        if rep_i > 0:
            nc.gpsimd.memset(vrep[:], 0.0)
        stot = round0(rep_i)
        tap("stot0", stot[:])
        squash_update(stot)
        tap("vrep0", vrep[:].rearrange("p c b -> p (c b)"))
        stot = round12(1, rep_i, last=False)
        tap("stot1", stot[:])
        squash_update(stot)
        tap("vrep1", vrep[:].rearrange("p c b -> p (c b)"))
        round12(2, rep_i, last=True)


TAP_SHAPES = {
    "stot0": [DV, C * B], "stot1": [DV, C * B],
    "vrep0": [128, C * B], "vrep1": [128, C * B],
    "logits1": [G * B, C * NLG], "den1": [G * B, NLG],
    "coupt1": [NLG, C * G * B],
}

IN_SHAPES = {
    "wa": [128, C * NLG * DI],
    "wp": [128, NCH * C * DV],
    "up": [128, NCH * B],
    "u4": [G * B, NLG * DI],
    "rw": [NLG, 8 * 128],
    "isid": [128, DV],
    "rep16": [DV, G * 32],
}


def build_nc(with_taps=False, repeat=1):
    nc = bacc.Bacc("TRN2", target_bir_lowering=False, debug=False,
                   num_devices=NCORES)
    dins = {name: nc.dram_tensor(name, shape,
                                 BF16 if name == "wa" else F32,
                                 kind="ExternalInput")
            for name, shape in IN_SHAPES.items()}
    # out keeps the [(c,d); b] flat layout: row = c*16+d
    out_d = nc.dram_tensor("out", [2 * 80, B], F32, kind="ExternalOutput")
    taps = None
    if with_taps:
        taps = {name: nc.dram_tensor(name, shape, F32, kind="ExternalOutput")
                for name, shape in TAP_SHAPES.items()}

    with tile.TileContext(nc) as tc, ExitStack() as ctx:
        _body(ctx, tc, dins, out_d, taps=taps, repeat=repeat)
    nc.compile()
    return nc


# --------------------------------------------------------------------------
# Host side
# --------------------------------------------------------------------------

def make_in_maps(x, W):
    x = np.ascontiguousarray(np.asarray(x, dtype=np.float32))
    W = np.ascontiguousarray(np.asarray(W, dtype=np.float32))
    u = x.reshape(B, N, DI)
    isid = np.tile(np.eye(DV, dtype=np.float32), (NCORES, 1))
    rep16 = np.zeros((DV, G, 32), np.float32)
    for d in range(DV):
        rep16[d, :, d] = 1.0
    rep16 = rep16.reshape(DV, G * 32)
    rwm = np.zeros((NLG, 8, 128), np.float32)
    for w in range(8):
        for n16 in range(16):
            rwm[w * 16 + n16, w, n16 * DI:(n16 + 1) * DI] = 1.0
    rwm = rwm.reshape(NLG, 8 * 128)

    in_maps = []
    for k in range(NCORES):
        sl = u[:, k * NL:(k + 1) * NL, :]                   # [B, 512, 8]
        Wk = W[k * NL:(k + 1) * NL]                         # [512, C, DI, DV]
        Wk_g = Wk.reshape(G, NLG, C, DI, DV)
        wa = np.zeros((G, 32, C, NLG, DI), np.float32)
        wa[:, :DV] = Wk_g.transpose(0, 4, 2, 1, 3)          # [g,d,c,nl,i]
        # packed: p = (n16, i), chunks of 16 n
        Wp = Wk.reshape(NCH, 16, C, DI, DV).transpose(1, 3, 0, 2, 4)
        # -> [n16, i, ch, c, d]
        Up = sl.reshape(B, NCH, 16, DI).transpose(2, 3, 1, 0)  # [n16,i,ch,b]
        u4 = sl.reshape(B, G, NLG, DI).transpose(1, 0, 2, 3)   # [g,b,nl,i]
        in_maps.append({
            "wa": np.ascontiguousarray(
                wa.reshape(128, C * NLG * DI)).astype(ml_dtypes.bfloat16),
            "wp": np.ascontiguousarray(Wp.reshape(128, NCH * C * DV)),
            "up": np.ascontiguousarray(Up.reshape(128, NCH * B)),
            "u4": np.ascontiguousarray(u4.reshape(G * B, NLG * DI)),
            "rw": rwm,
            "isid": isid,
            "rep16": rep16,
        })
    return in_maps


def postprocess(outs):
    """outs: list (per core) of [(c,d)=160, b] partials -> final [B, C, DV]."""
    s = np.zeros((C, DV, B), np.float64)
    for o in outs:
        s += o.reshape(C, DV, B).astype(np.float64)
    s = s.transpose(2, 0, 1)                                # [b, c, d]
    s2 = np.sum(s * s, axis=-1, keepdims=True)
    v = (s2 / (1.0 + s2) / np.sqrt(s2 + EPS)) * s
    return v.astype(np.float32)


_NC_CACHE = {}


def kernel(x, W):
    if "nc" not in _NC_CACHE:
        _NC_CACHE["nc"] = build_nc()
    nc = _NC_CACHE["nc"]
    in_maps = make_in_maps(x, W)
    res = run_bass_kernel_spmd(nc, in_maps, list(range(NCORES)))
    outs = [res.results[k]["out"] for k in range(NCORES)]
    return postprocess(outs)
